# revision 1
# baseline (speedup 1.0000x reference)
"""Trainium2 Bass kernel for nn_MultiHead (dense transformer layer).

Strategy: pure data-parallel over batch (B=8 -> 8 NeuronCores, no collectives).
Per core: full transformer layer on one [S=1024, D=1024] batch element.

Layout scheme:
  - Activations feeding matmuls are kept TRANSPOSED in SBUF ([feature partitions, seq free])
    so weights (host-pre-transposed to W^T) load with zero device transposes.
  - LayerNorm / softmax-application run in NATURAL layout ([seq partitions, feature free])
    where per-token stats are per-partition scalars; PE-transposes (128x128 identity
    matmuls) convert between the two.
  - Attention: scores^T per head via K^T/Q^T slices; softmax denominator comes free by
    augmenting V with a ones-column in the ctx matmul (row 64 of the psum = sum_t exp).
  - All big matmuls run as float32r (full-speed fp32 PE mode), fp32 accumulate in PSUM.

Tile pools are strictly LIFO, so long-lived pools are opened first and phase-local
pools are pushed/popped around each phase.
"""
from contextlib import ExitStack

import numpy as np

S = 1024
D = 1024
H = 16
DH = 64
DFF = 4096
P = 128
B = 8
NCORES = 8
EPS = 1e-8

_RUNNER = None


# ---------------------------------------------------------------- device kernel
def build_nc():
    import concourse.bass as bass
    import concourse.mybir as mybir
    import concourse.tile as tile
    from concourse import bacc
    from contextlib import ExitStack

    f32 = mybir.dt.float32
    f32r = mybir.dt.float32r
    AF = mybir.ActivationFunctionType
    ALU = mybir.AluOpType

    nc = bacc.Bacc("TRN2", target_bir_lowering=False, debug=False)

    # ---- I/O -----------------------------------------------------------------
    xT = nc.declare_dram_parameter("xT", [D, S], f32r, isOutput=False)
    xN = nc.declare_dram_parameter("xN", [S, D], f32, isOutput=False)
    wq = nc.declare_dram_parameter("wq", [8, P, 8, P], f32r, isOutput=False)
    wk = nc.declare_dram_parameter("wk", [8, P, 8, P], f32r, isOutput=False)
    wv = nc.declare_dram_parameter("wv", [P, 8, D], f32r, isOutput=False)
    wp = nc.declare_dram_parameter("wp", [P, 8, D], f32r, isOutput=False)
    wf1 = nc.declare_dram_parameter("wf1", [32, P, 8, P], f32r, isOutput=False)
    wf2 = nc.declare_dram_parameter("wf2", [32, P, D], f32r, isOutput=False)
    qb = nc.declare_dram_parameter("qb", [D], f32, isOutput=False)
    kb = nc.declare_dram_parameter("kb", [D], f32, isOutput=False)
    vb = nc.declare_dram_parameter("vb", [D], f32r, isOutput=False)
    f1b = nc.declare_dram_parameter("f1b", [DFF], f32, isOutput=False)
    f2b = nc.declare_dram_parameter("f2b", [D], f32r, isOutput=False)
    pb = nc.declare_dram_parameter("pb", [D], f32r, isOutput=False)
    lng = nc.declare_dram_parameter("lng", [D], f32, isOutput=False)
    lnb = nc.declare_dram_parameter("lnb", [D], f32, isOutput=False)
    fflng = nc.declare_dram_parameter("fflng", [D], f32, isOutput=False)
    fflnb = nc.declare_dram_parameter("fflnb", [D], f32, isOutput=False)
    idf = nc.declare_dram_parameter("idf", [P, P], f32, isOutput=False)
    idr = nc.declare_dram_parameter("idr", [P, P], f32r, isOutput=False)
    onesrow = nc.declare_dram_parameter("onesrow", [1, P], f32r, isOutput=False)
    y = nc.declare_dram_parameter("y", [S, D], f32, isOutput=True)

    def mm(out, lhsT, rhs, start, stop):
        nc.tensor.matmul(out, lhsT, rhs, start=start, stop=stop)

    with tile.TileContext(nc) as tc:
        es_consts = ExitStack()
        es_mid = ExitStack()

        consts = es_consts.enter_context(tc.tile_pool(name="consts", bufs=1))
        midp = es_mid.enter_context(tc.tile_pool(name="midp", bufs=1))

        identf = consts.tile([P, P], f32)
        nc.sync.dma_start(identf[:], idf[:])
        ones1 = consts.tile([1, P], f32r)
        nc.sync.dma_start(ones1[:], onesrow[:])
        eps_t = consts.tile([P, 1], f32)
        nc.vector.memset(eps_t[:], EPS)
        ones_pp = consts.tile([P, 1], f32)
        nc.vector.memset(ones_pp[:], 1.0)
        qb_sb = consts.tile([P, 8], f32)
        nc.sync.dma_start(qb_sb[:], qb[:].rearrange("(j p) -> p j", p=P))
        kb_sb = consts.tile([P, 8], f32)
        nc.sync.dma_start(kb_sb[:], kb[:].rearrange("(j p) -> p j", p=P))
        f1b_sb = consts.tile([P, 32], f32)
        nc.sync.dma_start(f1b_sb[:], f1b[:].rearrange("(j p) -> p j", p=P))

        # persistent mid-life tensors
        CN = midp.tile([P, 8, D], f32, tag="cnff")
        Dt = midp.tile([16, S], f32, tag="Dt")

        # ---- phase 1: QKV projections ----------------------------------------
        es_qkv = ExitStack()
        qkv = es_qkv.enter_context(tc.tile_pool(name="qkv", bufs=1))
        QT = qkv.tile([P, 8, S], f32r, tag="QT")
        KT = qkv.tile([P, 8, S], f32r, tag="KT")
        Vp = qkv.tile([P, 8, H * (DH + 1)], f32r, tag="Vp")
        Vp5 = Vp[:].rearrange("p i (hh e) -> p i hh e", e=DH + 1)

        es_p1 = ExitStack()
        xt_pool = es_p1.enter_context(tc.tile_pool(name="xtp", bufs=1))
        w1 = es_p1.enter_context(tc.tile_pool(name="w1", bufs=2))
        wvs = es_p1.enter_context(tc.tile_pool(name="wvs", bufs=3))
        ps1p = es_p1.enter_context(tc.tile_pool(name="ps1", bufs=8, space="PSUM"))

        XT = xt_pool.tile([P, 8, S], f32r)
        xTr = xT[:].rearrange("(ko p) s -> p ko s", p=P)
        for k in range(8):
            nc.sync.dma_start(XT[:, k, :], xTr[:, k, :])
        vb_row = xt_pool.tile([1, D], f32r)
        nc.sync.dma_start(vb_row[:], vb[None, :])

        for wdram, bias_sb, out in ((wq, qb_sb, QT), (wk, kb_sb, KT)):
            for j in range(8):
                wj = w1.tile([P, 8, P], f32r, tag="wqk")
                nc.sync.dma_start(wj[:], wdram[j])
                for c in range(2):
                    pt = ps1p.tile([P, 512], f32, tag="ps1")
                    for k in range(8):
                        mm(pt[:], wj[:, k, :], XT[:, k, c * 512:(c + 1) * 512],
                           start=(k == 0), stop=(k == 7))
                    nc.scalar.activation(out[:, j, c * 512:(c + 1) * 512], pt[:],
                                         AF.Relu, bias=bias_sb[:, j:j + 1])

        # V natural, packed per head with trailing ones column
        vp_col = Vp[:].rearrange("p i (hh e) -> p (i hh) e", e=DH + 1)[:, :, DH]
        nc.scalar.activation(vp_col, ones_pp[:].to_broadcast((P, 8 * H)), AF.Copy)
        for c in range(2):
            pv = [ps1p.tile([P, 512], f32, tag="ps1", name=f"pv_{c}_{i}")
                  for i in range(8)]
            for k in range(8):
                wvt = wvs.tile([P, 512], f32r, tag="wvt")
                nc.sync.dma_start(wvt[:], wv[:, k, c * 512:(c + 1) * 512])
                for i in range(8):
                    mm(pv[i][:], XT[:, k, i * 128:(i + 1) * 128], wvt[:],
                       start=(k == 0), stop=False)
            for i in range(8):
                mm(pv[i][:], ones1[:], vb_row[:, c * 512:(c + 1) * 512],
                   start=False, stop=True)
                nc.scalar.activation(Vp5[:, i, c * 8:(c + 1) * 8, 0:DH], pv[i][:],
                                     AF.Relu)

        es_p1.close()

        # ---- phase 2: attention ----------------------------------------------
        es_p2 = ExitStack()
        ctjp = es_p2.enter_context(tc.tile_pool(name="ctjp", bufs=2))
        e_pool = es_p2.enter_context(tc.tile_pool(name="ep", bufs=4))
        ds_pool = es_p2.enter_context(tc.tile_pool(name="dsp", bufs=2))
        ps2p = es_p2.enter_context(tc.tile_pool(name="ps2", bufs=2, space="PSUM"))

        for j in range(8):
            ctj = ctjp.tile([P, S], f32, tag="ctj")
            for u in range(2):
                h = 2 * j + u
                r0 = 64 * u
                for c in range(2):
                    cs = slice(c * 512, (c + 1) * 512)
                    cp = ps2p.tile([P, 512], f32, tag="cx")
                    for t in range(8):
                        sp = ps2p.tile([P, 512], f32, tag="sc")
                        mm(sp[:], KT[r0:r0 + 64, j, t * 128:(t + 1) * 128],
                           QT[r0:r0 + 64, j, cs], start=True, stop=True)
                        et = e_pool.tile([P, 512], f32r, tag="E")
                        nc.scalar.activation(et[:], sp[:], AF.Exp, scale=0.125)
                        mm(cp[:65], Vp5[:, t, h, :], et[:],
                           start=(t == 0), stop=(t == 7))
                    nc.vector.tensor_copy(ctj[r0:r0 + 64, cs], cp[0:64])
                    ds = ds_pool.tile([1, 512], f32, tag="dstage")
                    nc.vector.tensor_copy(ds[:], cp[64:65])
                    nc.gpsimd.dma_start(Dt[h:h + 1, cs], ds[:])
            # transpose this d-tile of ctx^T into natural CN columns
            for i in range(8):
                pt = ps2p.tile([P, P], f32, tag="tr2")
                nc.tensor.transpose(pt[:], ctj[:, i * 128:(i + 1) * 128], identf[:])
                nc.scalar.activation(CN[:, i, j * 128:(j + 1) * 128], pt[:], AF.Copy)

        es_p2.close()
        es_qkv.close()

        # ---- phase 3: softmax-normalize + residual + LN1 + transpose ---------
        es_o1t = ExitStack()
        o1tp = es_o1t.enter_context(tc.tile_pool(name="o1tp", bufs=1))
        O1T = o1tp.tile([P, 8, S], f32r)

        es_p3 = ExitStack()
        ln1c = es_p3.enter_context(tc.tile_pool(name="ln1c", bufs=1))
        ln1p = es_p3.enter_context(tc.tile_pool(name="ln1p", bufs=2))
        ps3p = es_p3.enter_context(tc.tile_pool(name="ps3", bufs=4, space="PSUM"))

        lng_b = ln1c.tile([P, D], f32, tag="lngb")
        nc.gpsimd.dma_start(lng_b[:], lng[None, :].to_broadcast((P, D)))
        lnb_b = ln1c.tile([P, D], f32, tag="lnbb")
        nc.gpsimd.dma_start(lnb_b[:], lnb[None, :].to_broadcast((P, D)))

        RD = ln1p.tile([P, 8, 16], f32, tag="RD")
        for i in range(8):
            pt = ps3p.tile([P, 16], f32, tag="trd")
            nc.tensor.transpose(pt[:], Dt[:, i * 128:(i + 1) * 128],
                                identf[:16, :16])
            nc.vector.reciprocal(RD[:, i, :], pt[:])

        for i in range(8):
            cn3 = CN[:, i, :].rearrange("p (hh e) -> p hh e", e=DH)
            nc.vector.tensor_tensor(cn3[:], cn3[:],
                                    RD[:, i, :, None].to_broadcast((P, H, DH)),
                                    ALU.mult)
            xn = ln1p.tile([P, D], f32, tag="xn")
            nc.sync.dma_start(xn[:], xN[i * 128:(i + 1) * 128, :])
            a = CN[:, i, :]
            nc.vector.tensor_tensor(a[:], a[:], xn[:], ALU.add)
            stats = ln1p.tile([P, 2, 6], f32, tag="bnst")
            for sg in range(2):
                nc.vector.bn_stats(stats[:, sg, :], a[:, sg * 512:(sg + 1) * 512])
            mv = ln1p.tile([P, 2], f32, tag="bnagg")
            nc.vector.bn_aggr(mv[:], stats[:])
            rstd = ln1p.tile([P, 1], f32, tag="rstd")
            nc.scalar.activation(rstd[:], mv[:, 1:2], AF.Sqrt, bias=eps_t[:])
            nc.vector.reciprocal(rstd[:], rstd[:])
            o1i = ln1p.tile([P, D], f32, tag="o1i")
            nc.vector.tensor_scalar(o1i[:], a[:], mv[:, 0:1], rstd[:],
                                    op0=ALU.subtract, op1=ALU.mult)
            nc.vector.tensor_tensor(o1i[:], o1i[:], lng_b[:], ALU.mult)
            nc.vector.tensor_tensor(o1i[:], o1i[:], lnb_b[:], ALU.add)
            for k in range(8):
                pt = ps3p.tile([P, P], f32, tag="tr")
                nc.tensor.transpose(pt[:], o1i[:, k * 128:(k + 1) * 128], identf[:])
                nc.scalar.activation(O1T[:, k, i * 128:(i + 1) * 128], pt[:],
                                     AF.Copy)

        es_p3.close()

        # ---- phase 4: feed-forward (seq halves) ------------------------------
        es_p4 = ExitStack()
        ffp = es_p4.enter_context(tc.tile_pool(name="ffp", bufs=1))
        wf1p = es_p4.enter_context(tc.tile_pool(name="wf1p", bufs=3))
        wf2p = es_p4.enter_context(tc.tile_pool(name="wf2p", bufs=3))
        ps4p = es_p4.enter_context(tc.tile_pool(name="ps4", bufs=8, space="PSUM"))

        f2b_row = ffp.tile([1, D], f32r, tag="f2brow")
        nc.sync.dma_start(f2b_row[:], f2b[None, :])

        FF = midp.tile([P, 8, D], f32, tag="cnff")

        for c in range(2):
            cs = slice(c * 512, (c + 1) * 512)
            H1 = ffp.tile([P, 32, 512], f32r, tag="h1")
            for m in range(32):
                wm = wf1p.tile([P, 8, P], f32r, tag="wf1")
                nc.sync.dma_start(wm[:], wf1[m])
                pt = ps4p.tile([P, 512], f32, tag="ff")
                for k in range(8):
                    mm(pt[:], wm[:, k, :], O1T[:, k, cs],
                       start=(k == 0), stop=(k == 7))
                nc.scalar.activation(H1[:, m, :], pt[:], AF.Relu,
                                     bias=f1b_sb[:, m:m + 1])
            pts = [[ps4p.tile([P, 512], f32, tag="ff", name=f"ffps_{c}_{ii}_{dh}")
                    for dh in range(2)] for ii in range(4)]
            for m in range(32):
                w2 = wf2p.tile([P, D], f32r, tag="wf2")
                nc.sync.dma_start(w2[:], wf2[m])
                for ii in range(4):
                    for dh in range(2):
                        mm(pts[ii][dh][:], H1[:, m, ii * 128:(ii + 1) * 128],
                           w2[:, dh * 512:(dh + 1) * 512],
                           start=(m == 0), stop=False)
            for ii in range(4):
                i = c * 4 + ii
                for dh in range(2):
                    ds_ = slice(dh * 512, (dh + 1) * 512)
                    mm(pts[ii][dh][:], ones1[:], f2b_row[:, ds_],
                       start=False, stop=True)
                    nc.scalar.activation(FF[:, i, ds_], pts[ii][dh][:], AF.Copy)

        es_p4.close()

        # ---- phase 5: LN2 + output projection --------------------------------
        es_p5 = ExitStack()
        latep = es_p5.enter_context(tc.tile_pool(name="latep", bufs=1))
        ln2p = es_p5.enter_context(tc.tile_pool(name="ln2p", bufs=2))
        ps5p = es_p5.enter_context(tc.tile_pool(name="ps5", bufs=3, space="PSUM"))

        identr = latep.tile([P, P], f32r, tag="identr")
        nc.sync.dma_start(identr[:], idr[:])
        fflng_b = latep.tile([P, D], f32, tag="fflngb")
        nc.gpsimd.dma_start(fflng_b[:], fflng[None, :].to_broadcast((P, D)))
        fflnb_b = latep.tile([P, D], f32, tag="fflnbb")
        nc.gpsimd.dma_start(fflnb_b[:], fflnb[None, :].to_broadcast((P, D)))
        pb_row = latep.tile([1, D], f32r, tag="pbrow")
        nc.sync.dma_start(pb_row[:], pb[None, :])
        WP = latep.tile([P, 8, D], f32r, tag="wp")
        for k in range(8):
            nc.sync.dma_start(WP[:, k, :], wp[:, k, :])
        O2T = latep.tile([P, 8, S], f32r, tag="O2T")

        for i in range(8):
            # reconstruct natural out1 from O1T
            o1n = ln2p.tile([P, D], f32, tag="o1n")
            for k in range(8):
                ptr = ps5p.tile([P, P], f32r, tag="tr5")
                nc.tensor.transpose(ptr[:], O1T[:, k, i * 128:(i + 1) * 128],
                                    identr[:])
                nc.scalar.activation(o1n[:, k * 128:(k + 1) * 128], ptr[:], AF.Copy)
            a = FF[:, i, :]
            nc.vector.tensor_tensor(a[:], a[:], o1n[:], ALU.add)
            stats = ln2p.tile([P, 2, 6], f32, tag="bnst2")
            for sg in range(2):
                nc.vector.bn_stats(stats[:, sg, :], a[:, sg * 512:(sg + 1) * 512])
            mv = ln2p.tile([P, 2], f32, tag="bnagg2")
            nc.vector.bn_aggr(mv[:], stats[:])
            rstd = ln2p.tile([P, 1], f32, tag="rstd2")
            nc.scalar.activation(rstd[:], mv[:, 1:2], AF.Sqrt, bias=eps_t[:])
            nc.vector.reciprocal(rstd[:], rstd[:])
            o2i = ln2p.tile([P, D], f32, tag="o2i")
            nc.vector.tensor_scalar(o2i[:], a[:], mv[:, 0:1], rstd[:],
                                    op0=ALU.subtract, op1=ALU.mult)
            nc.vector.tensor_tensor(o2i[:], o2i[:], fflng_b[:], ALU.mult)
            nc.vector.tensor_tensor(o2i[:], o2i[:], fflnb_b[:], ALU.add)
            for k in range(8):
                pt = ps5p.tile([P, P], f32, tag="tr5")
                nc.tensor.transpose(pt[:], o2i[:, k * 128:(k + 1) * 128], identf[:])
                nc.scalar.activation(O2T[:, k, i * 128:(i + 1) * 128], pt[:],
                                     AF.Copy)
            yt = ln2p.tile([P, D], f32, tag="yst")
            for dh in range(2):
                ds_ = slice(dh * 512, (dh + 1) * 512)
                pt = ps5p.tile([P, 512], f32, tag="pr")
                for k in range(8):
                    mm(pt[:], O2T[:, k, i * 128:(i + 1) * 128], WP[:, k, ds_],
                       start=(k == 0), stop=False)
                mm(pt[:], ones1[:], pb_row[:, ds_], start=False, stop=True)
                nc.scalar.activation(yt[:, ds_], pt[:], AF.Copy)
            nc.sync.dma_start(y[i * 128:(i + 1) * 128, :], yt[:])

        es_p5.close()
        es_o1t.close()
        es_mid.close()
        es_consts.close()

    nc.compile()
    return nc


# ---------------------------------------------------------------- host wrapper
class _SpmdRunner:
    """Compile once, run repeatedly (mirrors bass2jax.run_bass_via_pjrt)."""

    def __init__(self, nc, n_cores):
        import jax
        from jax.sharding import Mesh, PartitionSpec
        from jax.experimental.shard_map import shard_map
        import concourse.mybir as mybir
        from concourse import bass2jax
        from concourse.bass2jax import _bass_exec_p, install_neuronx_cc_hook

        install_neuronx_cc_hook()
        self.n_cores = n_cores
        partition_name = (
            nc.partition_id_tensor.name if nc.partition_id_tensor else None
        )
        in_names, out_names, out_avals, zero_outs = [], [], [], []
        for alloc in nc.m.functions[0].allocations:
            if not isinstance(alloc, mybir.MemoryLocationSet):
                continue
            name = alloc.memorylocations[0].name
            if alloc.kind == "ExternalInput":
                if name != partition_name:
                    in_names.append(name)
            elif alloc.kind == "ExternalOutput":
                shape = tuple(alloc.tensor_shape)
                dtype = mybir.dt.np(alloc.dtype)
                out_names.append(name)
                out_avals.append(jax.core.ShapedArray(shape, dtype))
                zero_outs.append(np.zeros(shape, dtype))
        self.in_names = in_names
        self.out_names = out_names
        self.out_avals = out_avals
        self.zero_outs = zero_outs
        n_params = len(in_names)
        n_outs = len(out_avals)
        all_in_names = in_names + out_names
        if partition_name is not None:
            all_in_names.append(partition_name)
        donate = tuple(range(n_params, n_params + n_outs))

        def _body(*args):
            operands = list(args)
            if partition_name is not None:
                operands.append(bass2jax.partition_id_tensor())
            outs = _bass_exec_p.bind(
                *operands,
                out_avals=tuple(out_avals),
                in_names=tuple(all_in_names),
                out_names=tuple(out_names),
                lowering_input_output_aliases=(),
                sim_require_finite=True,
                sim_require_nnan=True,
                nc=nc,
            )
            return tuple(outs)

        import jax as _jax
        devices = _jax.devices()[:n_cores]
        assert len(devices) == n_cores
        mesh = Mesh(np.asarray(devices), ("core",))
        in_specs = (PartitionSpec("core"),) * (n_params + n_outs)
        out_specs = (PartitionSpec("core"),) * n_outs
        self.fn = _jax.jit(
            shard_map(_body, mesh=mesh, in_specs=in_specs,
                      out_specs=out_specs, check_rep=False),
            donate_argnums=donate,
            keep_unused=True,
        )

    def prep_inputs(self, in_maps):
        per_core = [[np.asarray(m[n]) for n in self.in_names] for m in in_maps]
        return [
            np.concatenate([per_core[c][i] for c in range(self.n_cores)], axis=0)
            for i in range(len(self.in_names))
        ]

    def zeros(self):
        return [
            np.zeros((self.n_cores * z.shape[0], *z.shape[1:]), z.dtype)
            for z in self.zero_outs
        ]

    def run_device(self, concat_in):
        return self.fn(*concat_in, *self.zeros())

    def split(self, out_arrs):
        return [
            {
                name: np.asarray(out_arrs[i]).reshape(
                    self.n_cores, *self.out_avals[i].shape)[c]
                for i, name in enumerate(self.out_names)
            }
            for c in range(self.n_cores)
        ]


def make_in_maps(**inputs):
    q = np.ascontiguousarray(np.asarray(inputs["queries"], dtype=np.float32))
    f32 = np.float32

    def arr(name):
        return np.ascontiguousarray(np.asarray(inputs[name], dtype=f32))

    Qw, Kw, Vw = arr("Qw"), arr("Kw"), arr("Vw")
    proj_w, ff1_w, ff2_w = arr("proj_w"), arr("ff1_w"), arr("ff2_w")

    # packed weight layouts (all-contiguous device DMAs)
    def pack_lhsT(w, nj):  # [dout, din] -> [j, p(k), ko, mc]
        return np.ascontiguousarray(
            w.reshape(nj, P, 8, P).transpose(0, 3, 2, 1))

    def pack_rhs(w):  # [dout, din] -> W^T as [p(k), ko, dout]
        return np.ascontiguousarray(
            w.T.reshape(8, P, w.shape[0]).transpose(1, 0, 2))

    shared = {
        "wq": pack_lhsT(Qw, 8),
        "wk": pack_lhsT(Kw, 8),
        "wv": pack_rhs(Vw),
        "wp": pack_rhs(proj_w),
        "wf1": pack_lhsT(ff1_w, 32),
        "wf2": np.ascontiguousarray(ff2_w.T.reshape(32, P, D)),
        "qb": arr("Qb"), "kb": arr("Kb"), "vb": arr("Vb"),
        "f1b": arr("ff1_b"), "f2b": arr("ff2_b"), "pb": arr("proj_b"),
        "lng": arr("ln_g"), "lnb": arr("ln_b"),
        "fflng": arr("ffln_g"), "fflnb": arr("ffln_b"),
        "idf": np.eye(P, dtype=f32), "idr": np.eye(P, dtype=f32),
        "onesrow": np.ones((1, P), dtype=f32),
    }
    in_maps = []
    for b in range(B):
        m = dict(shared)
        m["xN"] = np.ascontiguousarray(q[b])
        m["xT"] = np.ascontiguousarray(q[b].T)
        in_maps.append(m)
    return in_maps


def get_runner():
    global _RUNNER
    if _RUNNER is None:
        nc = build_nc()
        _RUNNER = _SpmdRunner(nc, NCORES)
    return _RUNNER


def kernel(**inputs):
    runner = get_runner()
    in_maps = make_in_maps(**inputs)
    res = runner.split(runner.run_device(runner.prep_inputs(in_maps)))
    out = np.stack([res[c]["y"] for c in range(NCORES)], axis=0)
    return out.astype(np.float32)

